# revision 8
# baseline (speedup 1.0000x reference)
"""2-layer GraphSAGE (mean aggregation) on 8 trn2 NeuronCores via Bass/Tile.

Strategy (v2):
  - Nodes row-sharded across 8 cores (6250 rows -> 6272 padded, 49 blocks of
    128); edges partitioned by destination core. Per 128-dst-node block the
    segment-mean is a one-hot matmul on the tensor engine:
        aggT[64f, 128d] += msgs[128e, 64f].T @ oh[128e, 128d]
    where oh[e, d] = (d == dst_local[e]) * (1/deg[dst[e]]) is built in ONE
    DVE tensor_scalar op (op0=is_equal, op1=mult with the per-edge 1/deg
    column) -- the mean scaling costs nothing extra.
  - Layer-1 messages x[src] are host-pregathered into edge-major tiles
    msg1[128, T1, 64] bf16 (64 real feature columns only) and streamed with
    large sequential DMAs.
  - h rows are PACKED two-nodes-per-256B-row (hf[i] = [h[2i], h[2i+1]]),
    halving AllGather bytes. Layer-2 tiles are split by src-row parity so
    each tile's matmul reads either cols 0:64 or 64:128 of the gathered
    256B rows (dma_gather's elem_size floor).
  - h ships in THREE region AllGathers (blocks [0,20)/[20,36)/[36,49)), each
    as soon as its dense groups finish; region-r layer-2 gathers are placed
    between collective r and collective r+1 on the Pool queue so they overlap
    the rest of layer 1. int16 gather positions: 8*20*128/2 = 10240 < 32768.
  - Layer-2 aggregation runs REGION-OUTER: per (block, region) partial sums
    accumulate via PSUM then add into an SBUF accumulator, so no PSUM bank
    waits for the last collective; the final region interleaves with the
    dense phase per group.
  - The whole gather index table is preloaded to SBUF once (no per-call idx
    DMAs); h/out stores issue from the Act queue right after the copies that
    produce them, keeping every engine queue hazard-free in program order.
"""

import numpy as np
import ml_dtypes

import concourse.bacc as bacc
import concourse.mybir as mybir
import concourse.tile as tile
from concourse.bass_utils import run_bass_kernel_spmd

P = 128
D = 64
F32 = mybir.dt.float32
BF16 = mybir.dt.bfloat16
I16 = mybir.dt.int16
BF = ml_dtypes.bfloat16

GCOL = 512          # dense-phase group width (one PSUM bank), 4 blocks
BPG = GCOL // P     # blocks per dense group


class Cfg:
    def __init__(self, N, n_cores=8, chunk=64, chunk2=16, msgs_bufs=4,
                 msgs2_bufs=10, region_blocks=(20, 16, 13)):
        assert N % n_cores == 0
        self.N = N
        self.n_cores = n_cores
        self.n_own = N // n_cores
        self.nblk = -(-self.n_own // P)
        self.n_own_pad = self.nblk * P
        assert sum(region_blocks) == self.nblk
        self.region_blocks = region_blocks
        self.nreg = len(region_blocks)
        self.reg_start_blk = np.concatenate([[0], np.cumsum(region_blocks)])
        # packed (2 nodes / row) gather positions must fit int16
        for nb in region_blocks:
            assert n_cores * nb * P // 2 < 32768
        self.chunk = chunk
        self.chunk2 = chunk2
        self.msgs_bufs = msgs_bufs
        self.msgs2_bufs = msgs2_bufs


class Meta:
    pass


def _wrap16(v):
    """slot i -> [i % 16, i // 16] layout used by dma_gather idx tables."""
    assert v.shape[0] % 16 == 0
    return np.ascontiguousarray(v.reshape(-1, 16).T)


def preprocess(edge_index, cfg):
    """Partition/group edges; build per-core gather idx + onehot tables."""
    src = np.asarray(edge_index[0], dtype=np.int64)
    dst = np.asarray(edge_index[1], dtype=np.int64)
    E = src.shape[0]
    NC, NBLK, NR = cfg.n_cores, cfg.nblk, cfg.nreg

    cnt = np.bincount(dst, minlength=cfg.N).astype(np.float32)
    inv = (1.0 / np.maximum(cnt, 1.0)).astype(np.float32)

    core = dst // cfg.n_own
    dstl = dst - core * cfg.n_own
    blk = dstl // P
    inb = (dstl - blk * P).astype(np.float32)
    inve = inv[dst]                      # per-edge 1/deg(dst)

    core_s = src // cfg.n_own
    r_in = src - core_s * cfg.n_own
    sblk = r_in // P
    region = np.searchsorted(cfg.reg_start_blk, sblk, side="right") - 1
    reg_rows = np.asarray(cfg.region_blocks) * P
    # packed row position inside hf_region  (2 local rows per 256B row)
    roff = r_in - cfg.reg_start_blk[region] * P
    pos = core_s * (reg_rows[region] // 2) + roff // 2
    parity = (roff % 2).astype(np.int64)

    meta = Meta()
    meta.cfg = cfg

    # ---------------- layer 1: single-region tiling --------------------
    key1 = core * NBLK + blk
    g1 = np.bincount(key1, minlength=NC * NBLK).reshape(NC, NBLK)
    t1_blk = np.maximum(1, -(-g1.max(axis=0) // P))          # tiles per block
    off1 = np.concatenate([[0], np.cumsum(t1_blk)])
    T1 = int(off1[-1])
    order = np.argsort(key1, kind="stable")
    gstart = np.concatenate([[0], np.cumsum(g1.reshape(-1))])[:-1]
    rank1 = np.empty(E, dtype=np.int64)
    rank1[order] = np.arange(E) - gstart[key1[order]]
    slot1 = off1[blk] * P + rank1                             # slot in [T1*P)

    meta.T1 = T1
    meta.block_tiles1 = [list(range(int(off1[b]), int(off1[b + 1])))
                         for b in range(NBLK)]

    # ---------------- layer 2: (block, region, parity) tiling ----------
    key2 = ((core * NBLK + blk) * NR + region) * 2 + parity
    ng2 = NC * NBLK * NR * 2
    g2 = np.bincount(key2, minlength=ng2).reshape(NC, NBLK * NR * 2)
    t2_grp = -(-g2.max(axis=0) // P)                          # [NBLK*NR*2]
    # tile ids ordered region-major: all region 0 tiles (by block), then r1..
    t2_grp_r = t2_grp.reshape(NBLK, NR, 2)
    tiles2 = []          # (block, region, parity) per tile, region-major
    meta.reg_tile_off = []
    for r in range(NR):
        meta.reg_tile_off.append(len(tiles2))
        for b in range(NBLK):
            for p_ in range(2):
                tiles2 += [(b, r, p_)] * int(t2_grp_r[b, r, p_])
    T2 = len(tiles2)
    meta.T2 = T2
    meta.tiles2 = tiles2
    meta.reg_tile_off.append(T2)
    # slot base per (b, r, p) group
    base2 = np.zeros((NBLK, NR, 2), np.int64)
    acc = 0
    for r in range(NR):
        for b in range(NBLK):
            for p_ in range(2):
                base2[b, r, p_] = acc
                acc += int(t2_grp_r[b, r, p_])
    order2 = np.argsort(key2, kind="stable")
    gstart2 = np.concatenate([[0], np.cumsum(g2.reshape(-1))])[:-1]
    rank2 = np.empty(E, dtype=np.int64)
    rank2[order2] = np.arange(E) - gstart2[key2[order2]]
    slot2 = base2[blk, region, parity] * P + rank2

    # per (region, block): list of (tile_id, parity)
    meta.block_tiles2 = [[[] for _ in range(NBLK)] for _ in range(NR)]
    for t, (b, r, p_) in enumerate(tiles2):
        meta.block_tiles2[r][b].append((t, p_))

    # ---------------- per-core tables ----------------------------------
    meta.dstf1, meta.invd1 = [], []     # [128, T1] f32
    meta.dstf2, meta.invd2 = [], []     # [128, T2] f32
    meta.idx2 = []                      # [128, T2*8] i16
    meta.slot_src = []                  # [T1*P] i64 (msg1 pregather)
    for k in range(NC):
        m = core == k
        s1 = slot1[m]
        df = np.full(T1 * P, -1.0, np.float32)
        iv = np.ones(T1 * P, np.float32)
        df[s1] = inb[m]
        iv[s1] = inve[m]
        meta.dstf1.append(np.ascontiguousarray(df.reshape(T1, P).T))
        meta.invd1.append(np.ascontiguousarray(iv.reshape(T1, P).T))

        ssrc = np.full(T1 * P, -1, np.int64)
        ssrc[s1] = src[m]
        meta.slot_src.append(ssrc)

        s2 = slot2[m]
        df = np.full(T2 * P, -1.0, np.float32)
        iv = np.ones(T2 * P, np.float32)
        df[s2] = inb[m]
        iv[s2] = inve[m]
        meta.dstf2.append(np.ascontiguousarray(df.reshape(T2, P).T))
        meta.invd2.append(np.ascontiguousarray(iv.reshape(T2, P).T))

        ix = np.zeros(T2 * P, np.int16)
        ix[s2] = pos[m]
        w = _wrap16(ix)                                   # [16, T2*8]
        meta.idx2.append(np.ascontiguousarray(np.tile(w, (8, 1))))

    # ---------------- DMA call lists -----------------------------------
    def chunks(t0, t1, csz):
        out = []
        t = t0
        while t < t1:
            out.append((t, min(csz, t1 - t)))
            t += csz
        return out

    meta.calls1 = chunks(0, T1, cfg.chunk)
    meta.calls2 = [chunks(meta.reg_tile_off[r], meta.reg_tile_off[r + 1],
                          cfg.chunk2) for r in range(NR)]
    return meta


def build_program(meta, parts=("gather", "agg", "dense", "store",
                               "collective"), reps=1, single_packet=False):
    cfg = meta.cfg
    NC, NBLK, NR = cfg.n_cores, cfg.nblk, cfg.nreg
    NP = cfg.n_own_pad
    T1, T2 = meta.T1, meta.T2
    nc = bacc.Bacc("TRN2", target_bir_lowering=False, debug=False,
                   num_devices=NC)

    g1 = "gather" in parts or "gather1" in parts
    g2 = "gather" in parts or "gather2" in parts

    msg1_dr = nc.dram_tensor("msg1", [P, T1, D], BF16, kind="ExternalInput")
    xoT_dr = nc.dram_tensor("xoT", [D, NP], F32, kind="ExternalInput")
    idx2_dr = nc.dram_tensor("idx2", [P, T2 * 8], I16, kind="ExternalInput")
    dstf1_dr = nc.dram_tensor("dstf1", [P, T1], F32, kind="ExternalInput")
    invd1_dr = nc.dram_tensor("invd1", [P, T1], F32, kind="ExternalInput")
    dstf2_dr = nc.dram_tensor("dstf2", [P, T2], F32, kind="ExternalInput")
    invd2_dr = nc.dram_tensor("invd2", [P, T2], F32, kind="ExternalInput")
    wl1_dr = nc.dram_tensor("wl1t", [D, D], F32, kind="ExternalInput")
    wr1_dr = nc.dram_tensor("wr1t", [D, D], F32, kind="ExternalInput")
    wl2_dr = nc.dram_tensor("wl2t", [D, D], F32, kind="ExternalInput")
    wr2_dr = nc.dram_tensor("wr2t", [D, D], F32, kind="ExternalInput")
    b1_dr = nc.dram_tensor("b1", [D, 1], F32, kind="ExternalInput")
    b2_dr = nc.dram_tensor("b2", [D, 1], F32, kind="ExternalInput")
    iota_dr = nc.dram_tensor("iota", [P, P], BF16, kind="ExternalInput")
    id_dr = nc.dram_tensor("ident", [D, D], F32, kind="ExternalInput")
    out_dr = nc.dram_tensor("out", [NP, D], F32, kind="ExternalOutput")

    reg_rows2 = [nb * P // 2 for nb in cfg.region_blocks]   # packed rows/core

    with tile.TileContext(nc) as tc:
        with (
            tc.tile_pool(name="const", bufs=1) as cpool,
            tc.tile_pool(name="big", bufs=1) as bpool,
            tc.tile_pool(name="msgs", bufs=cfg.msgs_bufs) as mpool,
            tc.tile_pool(name="msgs2", bufs=cfg.msgs2_bufs) as mpool2,
            tc.tile_pool(name="ohp", bufs=12) as ohpool,
            tc.tile_pool(name="grp", bufs=2) as gpool,
            tc.tile_pool(name="psG", bufs=2, space="PSUM") as psG,
            tc.tile_pool(name="psZ", bufs=2, space="PSUM") as psZ,
            tc.tile_pool(name="psT", bufs=2, space="PSUM") as psT,
            tc.tile_pool(name="dram", bufs=1, space="DRAM") as dpool,
        ):
            def load(pool, dr, shape, name, dt=F32):
                t = pool.tile(shape, dt, name=name, tag=name)
                nc.sync.dma_start(out=t, in_=dr.ap())
                return t

            iota_sb = load(cpool, iota_dr, [P, P], "iota_sb", dt=BF16)
            ident_sb = load(cpool, id_dr, [D, D], "ident_sb")
            wl1_sb = load(cpool, wl1_dr, [D, D], "wl1_sb")
            wr1_sb = load(cpool, wr1_dr, [D, D], "wr1_sb")
            wl2_sb = load(cpool, wl2_dr, [D, D], "wl2_sb")
            wr2_sb = load(cpool, wr2_dr, [D, D], "wr2_sb")
            b1_sb = load(cpool, b1_dr, [D, 1], "b1_sb")
            b2_sb = load(cpool, b2_dr, [D, 1], "b2_sb")
            dstf1_sb = load(bpool, dstf1_dr, [P, T1], "dstf1_sb")
            invd1_sb = load(bpool, invd1_dr, [P, T1], "invd1_sb")
            dstf2_sb = load(bpool, dstf2_dr, [P, T2], "dstf2_sb")
            invd2_sb = load(bpool, invd2_dr, [P, T2], "invd2_sb")
            idx2_sb = load(bpool, idx2_dr, [P, T2 * 8], "idx2_sb", dt=I16)
            xoT_sb = load(bpool, xoT_dr, [D, NP], "xoT_sb")
            hT_sb = bpool.tile([D, NP], F32, name="hT_sb")
            acc2_sb = bpool.tile([D, NBLK * P], F32, name="acc2_sb")
            nodeh_sb = bpool.tile([P, NBLK * D], BF16, name="nodeh_sb")
            nodeo_sb = bpool.tile([P, NBLK * D], F32, name="nodeo_sb")

            def onehot(layer, gt, name):
                oh = ohpool.tile([P, P], BF16, tag="oh", name=name)
                dstf = dstf1_sb if layer == 0 else dstf2_sb
                invd = invd1_sb if layer == 0 else invd2_sb
                nc.vector.tensor_scalar(
                    out=oh, in0=iota_sb,
                    scalar1=dstf[:, gt:gt + 1],
                    scalar2=invd[:, gt:gt + 1],
                    op0=mybir.AluOpType.is_equal,
                    op1=mybir.AluOpType.mult,
                )
                return oh

            ngrp = -(-NBLK // BPG)
            grp_of_blk = [b // BPG for b in range(NBLK)]
            # region r ships after dense group: last block of region
            reg_last_grp = [grp_of_blk[int(cfg.reg_start_blk[r + 1]) - 1]
                            for r in range(NR)]

            for rep in range(reps):
                hc = [dpool.tile([reg_rows2[r], P], BF16,
                                 name=f"hc{r}_{rep}", tag=f"hc{r}_{rep}")
                      for r in range(NR)]
                hf = [dpool.tile([NC * reg_rows2[r], P], BF16,
                                 name=f"hf{r}_{rep}", tag=f"hf{r}_{rep}",
                                 addr_space="Shared")
                      for r in range(NR)]

                # ---------------- layer 1 -------------------------------
                tsrc = {}
                for ci, (t0, ntile) in enumerate(meta.calls1):
                    mt = mpool.tile([P, cfg.chunk, D], BF16, tag="msgs",
                                    name=f"m1_{rep}_{ci}")
                    if g1:
                        nc.sync.dma_start(
                            out=mt[:, :ntile, :],
                            in_=msg1_dr.ap()[:, t0:t0 + ntile, :])
                    for j in range(ntile):
                        tsrc[t0 + j] = (mt, j)

                def ship_region(r):
                    # pack nodeh region slice into hc[r] and AllGather
                    b0 = int(cfg.reg_start_blk[r])
                    nb = cfg.region_blocks[r]
                    nc.scalar.dma_start(
                        out=hc[r].rearrange("(b p2) (q f) -> (p2 q) b f",
                                            p2=P // 2, q=2, f=D),
                        in_=nodeh_sb[:, b0 * D:(b0 + nb) * D]
                            .rearrange("p (b f) -> p b f", f=D),
                    )
                    if "collective" in parts:
                        nc.gpsimd.collective_compute(
                            "AllGather",
                            mybir.AluOpType.bypass,
                            replica_groups=[list(range(NC))],
                            ins=[hc[r].opt()],
                            outs=[hf[r].opt()],
                        )

                def gather_region(r):
                    out = []
                    for ci, (t0, ntile) in enumerate(meta.calls2[r]):
                        mt = mpool2.tile([P, cfg.chunk2, P], BF16,
                                         tag="msgs2", name=f"m2_{rep}_{r}_{ci}")
                        if g2:
                            nc.gpsimd.dma_gather(
                                mt[:, :ntile, :],
                                hf[r][:, :],
                                idx2_sb[:, t0 * 8:(t0 + ntile) * 8],
                                num_idxs=ntile * P,
                                num_idxs_reg=ntile * P,
                                elem_size=P,
                                single_packet=single_packet,
                            )
                        for j in range(ntile):
                            out.append((t0 + j, mt, j))
                    return out

                tsrc2 = {}
                for g in range(ngrp if "agg" in parts else 0):
                    b0 = g * BPG
                    nb = min(BPG, NBLK - b0)
                    w = nb * P
                    aggT = gpool.tile([D, GCOL], F32, tag="aggT",
                                      name=f"agg1_{rep}_{g}")
                    ps = psG.tile([D, GCOL], F32, tag="agg", name=f"ps1_{rep}_{g}")
                    for bi in range(nb):
                        b = b0 + bi
                        gts = meta.block_tiles1[b]
                        for j, gt in enumerate(gts):
                            oh = onehot(0, gt, f"oh1_{rep}_{b}_{j}")
                            mt, lt = tsrc[gt]
                            nc.tensor.matmul(
                                ps[:, bi * P:(bi + 1) * P],
                                lhsT=mt[:, lt, :], rhs=oh,
                                start=(j == 0), stop=(j == len(gts) - 1),
                            )
                    # PSUM -> SBUF (mean already applied via scaled onehot)
                    nc.scalar.copy(out=aggT[:, :w], in_=ps[:, :w])
                    if "dense" in parts:
                        zp = psZ.tile([D, GCOL], F32, tag="z", name=f"z1_{rep}_{g}")
                        nc.tensor.matmul(zp[:, :w], lhsT=wl1_sb, rhs=aggT[:, :w],
                                         start=True, stop=False)
                        nc.tensor.matmul(zp[:, :w], lhsT=wr1_sb,
                                         rhs=xoT_sb[:, b0 * P:b0 * P + w],
                                         start=False, stop=True)
                        nc.scalar.activation(
                            out=hT_sb[:, b0 * P:b0 * P + w], in_=zp[:, :w],
                            func=mybir.ActivationFunctionType.Tanh,
                            bias=b1_sb[:, 0:1], scale=1.0)
                        if "store" in parts:
                            for bi in range(nb):
                                b = b0 + bi
                                tp = psT.tile([P, D], F32, tag="tr",
                                              name=f"tp1_{rep}_{b}")
                                nc.tensor.transpose(
                                    out=tp, in_=hT_sb[:, b * P:b * P + P],
                                    identity=ident_sb)
                                nc.scalar.copy(
                                    out=nodeh_sb[:, b * D:(b + 1) * D], in_=tp)
                    if "store" in parts and "dense" in parts:
                        for r in range(NR):
                            if reg_last_grp[r] == g:
                                ship_region(r)
                                tsrc2[r] = gather_region(r)

                # ---------------- layer 2 -------------------------------
                def agg2_group(r, g, lut):
                    """Matmul-accumulate region-r tiles of group g into one
                    [64, GCOL] PSUM tile; returns (ps, per-block has-tiles)."""
                    b0 = g * BPG
                    nb = min(BPG, NBLK - b0)
                    ps = psG.tile([D, GCOL], F32, tag="p2",
                                  name=f"ps2_{rep}_{r}_{g}")
                    has = []
                    for bi in range(nb):
                        b = b0 + bi
                        tps = meta.block_tiles2[r][b]
                        has.append(bool(tps))
                        for j, (t, par) in enumerate(tps):
                            oh = onehot(1, t, f"oh2_{rep}_{r}_{b}_{j}")
                            mt, lt = lut[t]
                            nc.tensor.matmul(
                                ps[:, bi * P:(bi + 1) * P],
                                lhsT=mt[:, lt, par * D:(par + 1) * D],
                                rhs=oh, start=(j == 0),
                                stop=(j == len(tps) - 1))
                    return ps, has, nb

                if "agg" in parts and "dense" in parts and "store" in parts:
                    # regions 0..NR-2: accumulate partials into acc2_sb
                    for r in range(NR - 1):
                        lut = {t: (mt, j) for (t, mt, j) in tsrc2[r]}
                        first = (r == 0)
                        for g in range(ngrp):
                            b0 = g * BPG
                            ps, has, nb = agg2_group(r, g, lut)
                            acc_sl = slice(b0 * P, b0 * P + nb * P)
                            if all(has):
                                if first:
                                    nc.scalar.copy(out=acc2_sb[:, acc_sl],
                                                   in_=ps[:, :nb * P])
                                else:
                                    nc.vector.tensor_tensor(
                                        out=acc2_sb[:, acc_sl],
                                        in0=ps[:, :nb * P],
                                        in1=acc2_sb[:, acc_sl],
                                        op=mybir.AluOpType.add)
                            else:
                                for bi in range(nb):
                                    sl_a = slice((b0 + bi) * P, (b0 + bi + 1) * P)
                                    sl_p = slice(bi * P, (bi + 1) * P)
                                    if has[bi]:
                                        if first:
                                            nc.scalar.copy(
                                                out=acc2_sb[:, sl_a],
                                                in_=ps[:, sl_p])
                                        else:
                                            nc.vector.tensor_tensor(
                                                out=acc2_sb[:, sl_a],
                                                in0=ps[:, sl_p],
                                                in1=acc2_sb[:, sl_a],
                                                op=mybir.AluOpType.add)
                                    elif first:
                                        nc.vector.memset(acc2_sb[:, sl_a], 0.0)
                    # last region interleaved with dense per group
                    rl = NR - 1
                    lut = {t: (mt, j) for (t, mt, j) in tsrc2[rl]}
                    for g in range(ngrp):
                        b0 = g * BPG
                        nb = min(BPG, NBLK - b0)
                        w = nb * P
                        aggT = gpool.tile([D, GCOL], F32, tag="aggT",
                                          name=f"agg2_{rep}_{g}")
                        ps, has, _nb = agg2_group(rl, g, lut)
                        acc_sl = slice(b0 * P, b0 * P + w)
                        if all(has):
                            nc.vector.tensor_tensor(
                                out=aggT[:, :w], in0=ps[:, :w],
                                in1=acc2_sb[:, acc_sl],
                                op=mybir.AluOpType.add)
                        else:
                            for bi in range(nb):
                                sl_a = slice((b0 + bi) * P, (b0 + bi + 1) * P)
                                sl_p = slice(bi * P, (bi + 1) * P)
                                if has[bi]:
                                    nc.vector.tensor_tensor(
                                        out=aggT[:, sl_p], in0=ps[:, sl_p],
                                        in1=acc2_sb[:, sl_a],
                                        op=mybir.AluOpType.add)
                                else:
                                    nc.scalar.copy(out=aggT[:, sl_p],
                                                   in_=acc2_sb[:, sl_a])
                        zp = psZ.tile([D, GCOL], F32, tag="z", name=f"z2_{rep}_{g}")
                        nc.tensor.matmul(zp[:, :w], lhsT=wl2_sb, rhs=aggT[:, :w],
                                         start=True, stop=False)
                        nc.tensor.matmul(zp[:, :w], lhsT=wr2_sb,
                                         rhs=hT_sb[:, b0 * P:b0 * P + w],
                                         start=False, stop=True)
                        outT = gpool.tile([D, GCOL], F32, tag="outT",
                                          name=f"oT_{rep}_{g}")
                        nc.scalar.activation(
                            out=outT[:, :w], in_=zp[:, :w],
                            func=mybir.ActivationFunctionType.Identity,
                            bias=b2_sb[:, 0:1], scale=1.0)
                        for bi in range(nb):
                            b = b0 + bi
                            tp = psT.tile([P, D], F32, tag="tr",
                                          name=f"tp2_{rep}_{b}")
                            nc.tensor.transpose(
                                out=tp, in_=outT[:, bi * P:(bi + 1) * P],
                                identity=ident_sb)
                            nc.scalar.copy(
                                out=nodeo_sb[:, b * D:(b + 1) * D], in_=tp)
                    nc.scalar.dma_start(
                        out=out_dr.ap().rearrange("(b p) f -> p b f", p=P),
                        in_=nodeo_sb.rearrange("p (b f) -> p b f", f=D),
                    )

    nc.compile()
    return nc


def make_in_maps(meta, x, W_l1, b_l1, W_r1, W_l2, b_l2, W_r2):
    cfg = meta.cfg
    x = np.ascontiguousarray(np.asarray(x, dtype=np.float32))
    xb = x.astype(BF)
    iota = np.tile(np.arange(P, dtype=np.float32), (P, 1)).astype(BF)
    ident = np.eye(D, dtype=np.float32)
    common = {
        "wl1t": np.ascontiguousarray(np.asarray(W_l1, np.float32).T),
        "wr1t": np.ascontiguousarray(np.asarray(W_r1, np.float32).T),
        "wl2t": np.ascontiguousarray(np.asarray(W_l2, np.float32).T),
        "wr2t": np.ascontiguousarray(np.asarray(W_r2, np.float32).T),
        "b1": np.asarray(b_l1, np.float32).reshape(D, 1).copy(),
        "b2": np.asarray(b_l2, np.float32).reshape(D, 1).copy(),
        "iota": iota,
        "ident": ident,
    }
    in_maps = []
    for k in range(cfg.n_cores):
        xo = x[k * cfg.n_own:(k + 1) * cfg.n_own]
        xoT = np.zeros((D, cfg.n_own_pad), np.float32)
        xoT[:, :cfg.n_own] = xo.T
        ssrc = meta.slot_src[k]
        m1 = xb[np.clip(ssrc, 0, None)]
        m1[ssrc < 0] = 0
        msg1 = np.ascontiguousarray(
            m1.reshape(meta.T1, P, D).transpose(1, 0, 2))
        in_maps.append(dict(common, xoT=xoT, msg1=msg1,
                            dstf1=meta.dstf1[k], invd1=meta.invd1[k],
                            dstf2=meta.dstf2[k], invd2=meta.invd2[k],
                            idx2=meta.idx2[k]))
    return in_maps


_CACHE = {}
_LAST_RES = None


def kernel(x, edge_index, W_l1, b_l1, W_r1, W_l2, b_l2, W_r2):
    edge_index = np.asarray(edge_index)
    x = np.asarray(x)
    cfg = Cfg(x.shape[0])
    key = hash(edge_index.tobytes())
    if key in _CACHE:
        meta, nc = _CACHE[key]
    else:
        meta = preprocess(edge_index, cfg)
        nc = build_program(meta)
        _CACHE[key] = (meta, nc)
    in_maps = make_in_maps(meta, x, W_l1, b_l1, W_r1, W_l2, b_l2, W_r2)
    res = run_bass_kernel_spmd(nc, in_maps, core_ids=list(range(cfg.n_cores)))
    global _LAST_RES
    _LAST_RES = res
    out = np.concatenate(
        [res.results[k]["out"][:cfg.n_own] for k in range(cfg.n_cores)], axis=0
    )
    return out.astype(np.float32)
